# revision 4
# baseline (speedup 1.0000x reference)
"""BlockLinear kernel for Trainium2 (8 NeuronCores, SPMD).

y[b, g*512+o] = sum_i x[b, g*512+i] * W[g, o, i] + bias[g, o]

Sharding: one block g per core (expert parallelism). Each core computes
y_g = x_g @ W_g^T + b_g with x_g = x[:, g*512:(g+1)*512].

Per-core device kernel:
  - inputs: xT [512, 16384] (x_g transposed on host), wT [512, 512]
    (W_g^T = [in, out]), bias [128, 512] (replicated over partitions)
  - per 1024-row group: DMA x tiles in, 8x(4 accumulating fp32 matmuls
    [K=128, M=128, N=512]) into PSUM, DVE adds bias while copying
    PSUM->SBUF, DMA the [1024, 512] result out contiguously.
"""

import numpy as np

import concourse.bass as bass
import concourse.mybir as mybir
import concourse.tile as tile
from concourse import bacc
from concourse.bass_utils import run_bass_kernel_spmd
from concourse.vector_clock import ScopedClock

F32 = mybir.dt.float32

NB, BIN, BOUT = 8, 512, 512
BATCH = 16384
NCORES = 8
P = 128
KT = BIN // P  # 4 k-tiles per block
GROUP = 1024  # batch rows per group
NSUB = GROUP // P  # 8 m-subtiles per group
NGROUPS = BATCH // GROUP  # 16

_patched = False


def _patch_tile_drain():
    """Walrus in this container accepts only one sync-wait per InstDrain;
    split the tile-exit drain's waits across one drain instruction each."""
    global _patched
    if _patched:
        return
    _patched = True

    def _drain_and_barrier(self, tick_clock, wait_clock):
        nc = self.nc
        drain_inst = nc.sync.drain()
        wait_clock.add_sem_waits(
            drain_inst.ins, ScopedClock({None: tick_clock.global_clock})
        )
        si = drain_inst.ins.sync_info
        if si is not None and len(si.on_wait) > 1:
            waits = list(si.on_wait)
            updates = list(si.on_update)
            drain_inst.ins.sync_info = mybir.SyncInfo(
                on_wait=[waits[0]], on_update=updates
            )
            for w in waits[1:]:
                extra = nc.sync.drain()
                extra.ins.sync_info = mybir.SyncInfo(on_wait=[w], on_update=[])
        nc.all_engine_barrier()
        popped = nc._tile_sem_poison_stack.pop()
        assert popped is self._sem_poison
        nc.clear_and_free_semaphores(list(self.sems.allocated().values()))
        nc.all_engine_barrier()

    tile.TileContext._drain_and_barrier = _drain_and_barrier


_nc_cache = None


def _build():
    global _nc_cache
    if _nc_cache is not None:
        return _nc_cache
    _patch_tile_drain()

    nc = bacc.Bacc(None, target_bir_lowering=False)
    xT = nc.dram_tensor("xT", [BIN, BATCH], F32, kind="ExternalInput")
    wT = nc.dram_tensor("wT", [BIN, BOUT], F32, kind="ExternalInput")
    bias = nc.dram_tensor("bias", [P, BOUT], F32, kind="ExternalInput")
    y = nc.dram_tensor("y", [BATCH, BOUT], F32, kind="ExternalOutput")

    with tile.TileContext(nc) as tc:
        with (
            tc.tile_pool(name="const", bufs=1) as const,
            tc.tile_pool(name="xp", bufs=3) as xp,
            tc.tile_pool(name="yp", bufs=3) as yp,
            tc.tile_pool(name="ps", bufs=8, space="PSUM") as psp,
        ):
            wt = const.tile([P, KT, BOUT], F32)
            nc.sync.dma_start(wt[:], wT.rearrange("(t p) o -> p t o", p=P))
            bt = const.tile([P, BOUT], F32)
            nc.sync.dma_start(bt[:], bias[:])

            for mg in range(NGROUPS):
                xt = xp.tile([P, KT, GROUP], F32)
                nc.sync.dma_start(
                    xt[:],
                    xT[:, mg * GROUP : (mg + 1) * GROUP].rearrange(
                        "(t p) b -> p t b", p=P
                    ),
                )
                yt = yp.tile([P, NSUB, BOUT], F32)
                for ms in range(NSUB):
                    ps = psp.tile([P, BOUT], F32)
                    for k in range(KT):
                        nc.tensor.matmul(
                            ps[:],
                            xt[:, k, ms * P : (ms + 1) * P],
                            wt[:, k, :],
                            start=(k == 0),
                            stop=(k == KT - 1),
                        )
                    nc.vector.tensor_add(out=yt[:, ms, :], in0=ps[:], in1=bt[:])
                nc.sync.dma_start(
                    y[mg * GROUP : (mg + 1) * GROUP, :].rearrange(
                        "(s p) o -> p s o", p=P
                    ),
                    yt[:],
                )
    nc.compile()
    _nc_cache = nc
    return nc


LAST_RESULT = None


def kernel(x, W, b, trace=False, trace_kwargs=None):
    global LAST_RESULT
    x = np.asarray(x, dtype=np.float32)
    W = np.asarray(W, dtype=np.float32)
    b = np.asarray(b, dtype=np.float32)

    nc = _build()
    in_maps = []
    for g in range(NCORES):
        xT_g = np.ascontiguousarray(x[:, g * BIN : (g + 1) * BIN].T)
        wT_g = np.ascontiguousarray(W[g].T)
        bias_g = np.ascontiguousarray(
            np.broadcast_to(b[g][None, :], (P, BOUT))
        )
        in_maps.append({"xT": xT_g, "wT": wT_g, "bias": bias_g})

    kwargs = dict(trace_kwargs or {})
    res = run_bass_kernel_spmd(nc, in_maps, list(range(NCORES)), trace=trace, **kwargs)
    LAST_RESULT = res

    out = np.empty((BATCH, NB * BOUT), dtype=np.float32)
    for g in range(NCORES):
        out[:, g * BOUT : (g + 1) * BOUT] = res.results[g]["y"]
    return out


# revision 5
# speedup vs baseline: 3.2560x; 3.2560x over previous
"""BlockLinear kernel for Trainium2 (8 NeuronCores, SPMD).

y[b, g*512+o] = sum_i x[b, g*512+i] * W[g, o, i] + bias[g, o]

Sharding: one block g per core (expert parallelism). Each core computes
y_g = x_g @ W_g^T + b_g with x_g = x[:, g*512:(g+1)*512].

Per-core device kernel (compute scheme "f16", rel err ~4e-4 vs fp32):
  - inputs: xT [512, 16384] fp16 (x_g transposed + cast on host),
    wT [512, 512] fp16 (W_g^T = [in, out]), bias [128, 512] fp32
    (replicated over partitions)
  - per 2048-row group: DMA x tiles in, 16x(4 accumulating matmuls
    [K=128, M=128, N=512] at 1 cycle/row) into PSUM, DVE adds bias while
    copying PSUM->SBUF (cast to out dtype), DMA the [2048, 512] result
    out contiguously.

Schemes: "f16" (fp16 in/compute, fp16 out), "f16_f32out" (fp32 output
DMA), "f32r" (fp32 bytes, TF32-like compute, rel err ~1.5e-4), "f32"
(exact, 4x slower PE).
"""

import numpy as np

import concourse.bass as bass
import concourse.mybir as mybir
import concourse.tile as tile
from concourse import bacc
from concourse.bass_utils import run_bass_kernel_spmd
from concourse.vector_clock import ScopedClock

F32 = mybir.dt.float32

NB, BIN, BOUT = 8, 512, 512
BATCH = 16384
NCORES = 8
P = 128
KT = BIN // P  # 4 k-tiles per block

SCHEME = "f16"  # "f16" | "f16_f32out" | "f32r" | "f32"

_patched = False


def _patch_tile_drain():
    """Walrus in this container accepts only one sync-wait per InstDrain;
    split the tile-exit drain's waits across one drain instruction each."""
    global _patched
    if _patched:
        return
    _patched = True

    def _drain_and_barrier(self, tick_clock, wait_clock):
        nc = self.nc
        drain_inst = nc.sync.drain()
        wait_clock.add_sem_waits(
            drain_inst.ins, ScopedClock({None: tick_clock.global_clock})
        )
        si = drain_inst.ins.sync_info
        if si is not None and len(si.on_wait) > 1:
            waits = list(si.on_wait)
            updates = list(si.on_update)
            drain_inst.ins.sync_info = mybir.SyncInfo(
                on_wait=[waits[0]], on_update=updates
            )
            for w in waits[1:]:
                extra = nc.sync.drain()
                extra.ins.sync_info = mybir.SyncInfo(on_wait=[w], on_update=[])
        nc.all_engine_barrier()
        popped = nc._tile_sem_poison_stack.pop()
        assert popped is self._sem_poison
        nc.clear_and_free_semaphores(list(self.sems.allocated().values()))
        nc.all_engine_barrier()

    tile.TileContext._drain_and_barrier = _drain_and_barrier


_nc_cache = {}


def _scheme_dtypes(scheme):
    if scheme in ("f16", "f16_f32out"):
        in_dt = mybir.dt.float16
        out_dt = mybir.dt.float16 if scheme == "f16" else F32
        np_in = np.float16
    elif scheme == "f32r":
        in_dt, out_dt, np_in = mybir.dt.float32r, F32, np.float32
    elif scheme == "f32":
        in_dt, out_dt, np_in = F32, F32, np.float32
    else:
        raise ValueError(scheme)
    return in_dt, out_dt, np_in


def _build(scheme=SCHEME, group=2048):
    key = (scheme, group)
    if key in _nc_cache:
        return _nc_cache[key]
    _patch_tile_drain()
    in_dt, out_dt, _ = _scheme_dtypes(scheme)
    nsub = group // P
    ngroups = BATCH // group

    nc = bacc.Bacc(None, target_bir_lowering=False)
    xT = nc.dram_tensor("xT", [BIN, BATCH], in_dt, kind="ExternalInput")
    wT = nc.dram_tensor("wT", [BIN, BOUT], in_dt, kind="ExternalInput")
    bias = nc.dram_tensor("bias", [P, BOUT], F32, kind="ExternalInput")
    y = nc.dram_tensor("y", [BATCH, BOUT], out_dt, kind="ExternalOutput")

    with tile.TileContext(nc) as tc:
        with (
            tc.tile_pool(name="const", bufs=1) as const,
            tc.tile_pool(name="xp", bufs=3) as xp,
            tc.tile_pool(name="yp", bufs=3) as yp,
            tc.tile_pool(name="ps", bufs=8, space="PSUM") as psp,
        ):
            wt = const.tile([P, KT, BOUT], in_dt)
            nc.sync.dma_start(wt[:], wT.rearrange("(t p) o -> p t o", p=P))
            bt = const.tile([P, BOUT], F32)
            nc.sync.dma_start(bt[:], bias[:])

            for mg in range(ngroups):
                xt = xp.tile([P, KT, group], in_dt)
                nc.sync.dma_start(
                    xt[:],
                    xT[:, mg * group : (mg + 1) * group].rearrange(
                        "(t p) b -> p t b", p=P
                    ),
                )
                yt = yp.tile([P, nsub, BOUT], out_dt)
                for ms in range(nsub):
                    ps = psp.tile([P, BOUT], F32)
                    for k in range(KT):
                        nc.tensor.matmul(
                            ps[:],
                            xt[:, k, ms * P : (ms + 1) * P],
                            wt[:, k, :],
                            start=(k == 0),
                            stop=(k == KT - 1),
                        )
                    nc.vector.tensor_add(out=yt[:, ms, :], in0=ps[:], in1=bt[:])
                nc.sync.dma_start(
                    y[mg * group : (mg + 1) * group, :].rearrange(
                        "(s p) o -> p s o", p=P
                    ),
                    yt[:],
                )
    nc.compile()
    _nc_cache[key] = nc
    return nc


LAST_RESULT = None


def kernel(x, W, b, trace=False, scheme=SCHEME, group=2048, trace_kwargs=None):
    global LAST_RESULT
    x = np.asarray(x, dtype=np.float32)
    W = np.asarray(W, dtype=np.float32)
    b = np.asarray(b, dtype=np.float32)

    _, _, np_in = _scheme_dtypes(scheme)
    nc = _build(scheme, group)
    in_maps = []
    for g in range(NCORES):
        xT_g = np.ascontiguousarray(x[:, g * BIN : (g + 1) * BIN].T.astype(np_in))
        wT_g = np.ascontiguousarray(W[g].T.astype(np_in))
        bias_g = np.ascontiguousarray(np.broadcast_to(b[g][None, :], (P, BOUT)))
        in_maps.append({"xT": xT_g, "wT": wT_g, "bias": bias_g})

    kwargs = dict(trace_kwargs or {})
    res = run_bass_kernel_spmd(nc, in_maps, list(range(NCORES)), trace=trace, **kwargs)
    LAST_RESULT = res

    out = np.empty((BATCH, NB * BOUT), dtype=np.float32)
    for g in range(NCORES):
        out[:, g * BOUT : (g + 1) * BOUT] = res.results[g]["y"].astype(np.float32)
    return out


# revision 6
# speedup vs baseline: 3.2882x; 1.0099x over previous
"""BlockLinear kernel for Trainium2 (8 NeuronCores, SPMD).

y[b, g*512+o] = sum_i x[b, g*512+i] * W[g, o, i] + bias[g, o]

Sharding: one block g per core (expert parallelism). Each core computes
y_g = x_g @ W_g^T + b_g with x_g = x[:, g*512:(g+1)*512].

Per-core device kernel (compute scheme "f16", rel err ~4e-4 vs fp32):
  - inputs: xT [512, 16384] fp16 (x_g transposed + cast on host),
    wT [512, 512] fp16 (W_g^T = [in, out]), bias [128, 512] fp32
    (replicated over partitions)
  - per 2048-row group: DMA x tiles in, 16x(4 accumulating matmuls
    [K=128, M=128, N=512] at 1 cycle/row) into PSUM, DVE adds bias while
    copying PSUM->SBUF (cast to out dtype), DMA the [2048, 512] result
    out contiguously.

Schemes: "f16" (fp16 in/compute, fp16 out), "f16_f32out" (fp32 output
DMA), "f32r" (fp32 bytes, TF32-like compute, rel err ~1.5e-4), "f32"
(exact, 4x slower PE).
"""

import numpy as np

import concourse.bass as bass
import concourse.mybir as mybir
import concourse.tile as tile
from concourse import bacc
from concourse.bass_utils import run_bass_kernel_spmd
from concourse.vector_clock import ScopedClock

F32 = mybir.dt.float32

NB, BIN, BOUT = 8, 512, 512
BATCH = 16384
NCORES = 8
P = 128
KT = BIN // P  # 4 k-tiles per block

SCHEME = "f16"  # "f16" | "f16_f32out" | "f32r" | "f32"

_patched = False


def _patch_tile_drain():
    """Walrus in this container accepts only one sync-wait per InstDrain;
    split the tile-exit drain's waits across one drain instruction each."""
    global _patched
    if _patched:
        return
    _patched = True

    def _drain_and_barrier(self, tick_clock, wait_clock):
        nc = self.nc
        drain_inst = nc.sync.drain()
        wait_clock.add_sem_waits(
            drain_inst.ins, ScopedClock({None: tick_clock.global_clock})
        )
        si = drain_inst.ins.sync_info
        if si is not None and len(si.on_wait) > 1:
            waits = list(si.on_wait)
            updates = list(si.on_update)
            drain_inst.ins.sync_info = mybir.SyncInfo(
                on_wait=[waits[0]], on_update=updates
            )
            for w in waits[1:]:
                extra = nc.sync.drain()
                extra.ins.sync_info = mybir.SyncInfo(on_wait=[w], on_update=[])
        nc.all_engine_barrier()
        popped = nc._tile_sem_poison_stack.pop()
        assert popped is self._sem_poison
        nc.clear_and_free_semaphores(list(self.sems.allocated().values()))
        nc.all_engine_barrier()

    tile.TileContext._drain_and_barrier = _drain_and_barrier


_nc_cache = {}


def _scheme_dtypes(scheme):
    if scheme in ("f16", "f16_f32out"):
        in_dt = mybir.dt.float16
        out_dt = mybir.dt.float16 if scheme == "f16" else F32
        np_in = np.float16
    elif scheme == "f32r":
        in_dt, out_dt, np_in = mybir.dt.float32r, F32, np.float32
    elif scheme == "f32":
        in_dt, out_dt, np_in = F32, F32, np.float32
    else:
        raise ValueError(scheme)
    return in_dt, out_dt, np_in


def _groups(group):
    """Batch-row group sizes: small first group (matmuls start sooner) and
    small last group (tail waits on a small final output DMA)."""
    sizes = [256, 1792]
    body = BATCH - 2 * (256 + 1792)
    sizes += [group] * (body // group)
    rem = body % group
    if rem:
        sizes.append(rem)
    sizes += [1792, 256]
    assert sum(sizes) == BATCH
    return sizes


def _build(scheme=SCHEME, group=2048):
    key = (scheme, group)
    if key in _nc_cache:
        return _nc_cache[key]
    _patch_tile_drain()
    in_dt, out_dt, _ = _scheme_dtypes(scheme)

    nc = bacc.Bacc(None, target_bir_lowering=False)
    xT = nc.dram_tensor("xT", [BIN, BATCH], in_dt, kind="ExternalInput")
    wT = nc.dram_tensor("wT", [BIN, BOUT], in_dt, kind="ExternalInput")
    bias = nc.dram_tensor("bias", [P, BOUT], F32, kind="ExternalInput")
    y = nc.dram_tensor("y", [BATCH, BOUT], out_dt, kind="ExternalOutput")

    with tile.TileContext(nc) as tc:
        with (
            tc.tile_pool(name="const", bufs=1) as const,
            tc.tile_pool(name="xp", bufs=4) as xp,
            tc.tile_pool(name="yp", bufs=4) as yp,
            tc.tile_pool(name="ps", bufs=8, space="PSUM") as psp,
        ):
            wt = const.tile([P, KT, BOUT], in_dt)
            nc.sync.dma_start(wt[:], wT.rearrange("(t p) o -> p t o", p=P))
            bt = const.tile([P, BOUT], F32)

            row = 0
            for mg, gsz in enumerate(_groups(group)):
                nsub = gsz // P
                xt = xp.tile([P, KT, gsz], in_dt, tag="xt")
                nc.sync.dma_start(
                    xt[:],
                    xT[:, row : row + gsz].rearrange("(t p) b -> p t b", p=P),
                )
                if mg == 0:
                    # bias is first needed by the first DVE add; load it
                    # after the first x tile so matmuls start sooner
                    nc.sync.dma_start(bt[:], bias[:])
                yt = yp.tile([P, nsub, BOUT], out_dt, tag="yt")
                for ms in range(nsub):
                    ps = psp.tile([P, BOUT], F32)
                    for k in range(KT):
                        nc.tensor.matmul(
                            ps[:],
                            xt[:, k, ms * P : (ms + 1) * P],
                            wt[:, k, :],
                            start=(k == 0),
                            stop=(k == KT - 1),
                        )
                    nc.vector.tensor_add(out=yt[:, ms, :], in0=ps[:], in1=bt[:])
                nc.sync.dma_start(
                    y[row : row + gsz, :].rearrange("(s p) o -> p s o", p=P),
                    yt[:],
                )
                row += gsz
    nc.compile()
    _nc_cache[key] = nc
    return nc


LAST_RESULT = None


def kernel(x, W, b, trace=False, scheme=SCHEME, group=2048, trace_kwargs=None):
    global LAST_RESULT
    x = np.asarray(x, dtype=np.float32)
    W = np.asarray(W, dtype=np.float32)
    b = np.asarray(b, dtype=np.float32)

    _, _, np_in = _scheme_dtypes(scheme)
    nc = _build(scheme, group)
    in_maps = []
    for g in range(NCORES):
        xT_g = np.ascontiguousarray(x[:, g * BIN : (g + 1) * BIN].T.astype(np_in))
        wT_g = np.ascontiguousarray(W[g].T.astype(np_in))
        bias_g = np.ascontiguousarray(np.broadcast_to(b[g][None, :], (P, BOUT)))
        in_maps.append({"xT": xT_g, "wT": wT_g, "bias": bias_g})

    kwargs = dict(trace_kwargs or {})
    res = run_bass_kernel_spmd(nc, in_maps, list(range(NCORES)), trace=trace, **kwargs)
    LAST_RESULT = res

    out = np.empty((BATCH, NB * BOUT), dtype=np.float32)
    for g in range(NCORES):
        out[:, g * BOUT : (g + 1) * BOUT] = res.results[g]["y"].astype(np.float32)
    return out


# revision 7
# speedup vs baseline: 3.5288x; 1.0731x over previous
"""BlockLinear kernel for Trainium2 (8 NeuronCores, SPMD).

y[b, g*512+o] = sum_i x[b, g*512+i] * W[g, o, i] + bias[g, o]

Sharding: one block g per core (expert parallelism). Each core computes
y_g = x_g @ W_g^T + b_g with x_g = x[:, g*512:(g+1)*512].

Per-core device kernel (compute scheme "f16", rel err ~4e-4 vs fp32):
  - inputs: xT [512, 16384] fp16 (x_g transposed + cast on host),
    wT [512, 512] fp16 (W_g^T = [in, out]), bias [128, 512] fp32
    (replicated over partitions)
  - per 2048-row group: DMA x tiles in, 16x(4 accumulating matmuls
    [K=128, M=128, N=512] at 1 cycle/row) into PSUM, DVE adds bias while
    copying PSUM->SBUF (cast to out dtype), DMA the [2048, 512] result
    out contiguously.

Schemes: "f16" (fp16 in/compute, fp16 out), "f16_f32out" (fp32 output
DMA), "f32r" (fp32 bytes, TF32-like compute, rel err ~1.5e-4), "f32"
(exact, 4x slower PE).
"""

import numpy as np

import concourse.bass as bass
import concourse.mybir as mybir
import concourse.tile as tile
from concourse import bacc
from concourse.bass_utils import run_bass_kernel_spmd
from concourse.vector_clock import ScopedClock

F32 = mybir.dt.float32

NB, BIN, BOUT = 8, 512, 512
BATCH = 16384
NCORES = 8
P = 128
KT = BIN // P  # 4 k-tiles per block

SCHEME = "f16"  # "f16" | "f16_f32out" | "f32r" | "f32"

_patched = False


def _patch_tile_drain():
    """Walrus in this container accepts only one sync-wait per InstDrain;
    split the tile-exit drain's waits across one drain instruction each."""
    global _patched
    if _patched:
        return
    _patched = True

    def _drain_and_barrier(self, tick_clock, wait_clock):
        nc = self.nc
        drain_inst = nc.sync.drain()
        wait_clock.add_sem_waits(
            drain_inst.ins, ScopedClock({None: tick_clock.global_clock})
        )
        si = drain_inst.ins.sync_info
        if si is not None and len(si.on_wait) > 1:
            waits = list(si.on_wait)
            updates = list(si.on_update)
            drain_inst.ins.sync_info = mybir.SyncInfo(
                on_wait=[waits[0]], on_update=updates
            )
            for w in waits[1:]:
                extra = nc.sync.drain()
                extra.ins.sync_info = mybir.SyncInfo(on_wait=[w], on_update=[])
        nc.all_engine_barrier()
        popped = nc._tile_sem_poison_stack.pop()
        assert popped is self._sem_poison
        nc.clear_and_free_semaphores(list(self.sems.allocated().values()))
        nc.all_engine_barrier()

    tile.TileContext._drain_and_barrier = _drain_and_barrier


_nc_cache = {}


def _scheme_dtypes(scheme):
    if scheme in ("f16", "f16_f32out"):
        in_dt = mybir.dt.float16
        out_dt = mybir.dt.float16 if scheme == "f16" else F32
        np_in = np.float16
    elif scheme == "f32r":
        in_dt, out_dt, np_in = mybir.dt.float32r, F32, np.float32
    elif scheme == "f32":
        in_dt, out_dt, np_in = F32, F32, np.float32
    else:
        raise ValueError(scheme)
    return in_dt, out_dt, np_in


def _groups(group):
    """Batch-row group sizes: geometric ramp at the start (matmuls start on
    the first small tile and never starve while DMA builds runway) and small
    final groups (tail waits on a small final output DMA)."""
    head = [256, 512, 1024, 1536]
    tail = [1536, 1024, 256]
    body = BATCH - sum(head) - sum(tail)
    sizes = head + [group] * (body // group)
    rem = body % group
    if rem:
        sizes.append(rem)
    sizes += tail
    assert sum(sizes) == BATCH, sizes
    return sizes


def _build(scheme=SCHEME, group=2048):
    key = (scheme, group)
    if key in _nc_cache:
        return _nc_cache[key]
    _patch_tile_drain()
    in_dt, out_dt, _ = _scheme_dtypes(scheme)

    nc = bacc.Bacc(None, target_bir_lowering=False)
    xT = nc.dram_tensor("xT", [BIN, BATCH], in_dt, kind="ExternalInput")
    wT = nc.dram_tensor("wT", [BIN, BOUT], in_dt, kind="ExternalInput")
    bias = nc.dram_tensor("bias", [P, BOUT], F32, kind="ExternalInput")
    y = nc.dram_tensor("y", [BATCH, BOUT], out_dt, kind="ExternalOutput")

    with tile.TileContext(nc) as tc:
        with (
            tc.tile_pool(name="const", bufs=1) as const,
            tc.tile_pool(name="xp", bufs=4) as xp,
            tc.tile_pool(name="yp", bufs=4) as yp,
            tc.tile_pool(name="ps", bufs=8, space="PSUM") as psp,
        ):
            wt = const.tile([P, KT, BOUT], in_dt)
            nc.sync.dma_start(wt[:], wT.rearrange("(t p) o -> p t o", p=P))
            bt = const.tile([P, BOUT], F32)

            row = 0
            for mg, gsz in enumerate(_groups(group)):
                nsub = gsz // P
                xt = xp.tile([P, KT, gsz], in_dt, tag="xt")
                nc.sync.dma_start(
                    xt[:],
                    xT[:, row : row + gsz].rearrange("(t p) b -> p t b", p=P),
                )
                if mg == 0:
                    # bias is first needed by the first DVE add; load it
                    # after the first x tile so matmuls start sooner
                    nc.sync.dma_start(bt[:], bias[:])
                yt = yp.tile([P, nsub, BOUT], out_dt, tag="yt")
                for ms in range(nsub):
                    ps = psp.tile([P, BOUT], F32)
                    for k in range(KT):
                        nc.tensor.matmul(
                            ps[:],
                            xt[:, k, ms * P : (ms + 1) * P],
                            wt[:, k, :],
                            start=(k == 0),
                            stop=(k == KT - 1),
                        )
                    nc.vector.tensor_add(out=yt[:, ms, :], in0=ps[:], in1=bt[:])
                nc.sync.dma_start(
                    y[row : row + gsz, :].rearrange("(s p) o -> p s o", p=P),
                    yt[:],
                )
                row += gsz
    nc.compile()
    _nc_cache[key] = nc
    return nc


LAST_RESULT = None


def kernel(x, W, b, trace=False, scheme=SCHEME, group=2048, trace_kwargs=None):
    global LAST_RESULT
    x = np.asarray(x, dtype=np.float32)
    W = np.asarray(W, dtype=np.float32)
    b = np.asarray(b, dtype=np.float32)

    _, _, np_in = _scheme_dtypes(scheme)
    nc = _build(scheme, group)
    in_maps = []
    for g in range(NCORES):
        xT_g = np.ascontiguousarray(x[:, g * BIN : (g + 1) * BIN].T.astype(np_in))
        wT_g = np.ascontiguousarray(W[g].T.astype(np_in))
        bias_g = np.ascontiguousarray(np.broadcast_to(b[g][None, :], (P, BOUT)))
        in_maps.append({"xT": xT_g, "wT": wT_g, "bias": bias_g})

    kwargs = dict(trace_kwargs or {})
    res = run_bass_kernel_spmd(nc, in_maps, list(range(NCORES)), trace=trace, **kwargs)
    LAST_RESULT = res

    out = np.empty((BATCH, NB * BOUT), dtype=np.float32)
    for g in range(NCORES):
        out[:, g * BOUT : (g + 1) * BOUT] = res.results[g]["y"].astype(np.float32)
    return out


# revision 12
# speedup vs baseline: 3.5873x; 1.0166x over previous
"""BlockLinear kernel for Trainium2 (8 NeuronCores, SPMD).

y[b, g*512+o] = sum_i x[b, g*512+i] * W[g, o, i] + bias[g, o]

Sharding: one block g per core (expert parallelism). Each core computes
y_g = x_g @ W_g^T + b_g with x_g = x[:, g*512:(g+1)*512].

Per-core device kernel (compute scheme "f16", rel err ~4e-4 vs fp32):
  - inputs: xT [512, 16384] fp16 (x_g transposed + cast on host),
    wT [512, 512] fp16 (W_g^T = [in, out]), bias [128, 512] fp32
    (replicated over partitions)
  - per 2048-row group: DMA x tiles in, 16x(4 accumulating matmuls
    [K=128, M=128, N=512] at 1 cycle/row) into PSUM, DVE adds bias while
    copying PSUM->SBUF (cast to out dtype), DMA the [2048, 512] result
    out contiguously.

Schemes: "f16" (fp16 in/compute, fp16 out), "f16_f32out" (fp32 output
DMA), "f32r" (fp32 bytes, TF32-like compute, rel err ~1.5e-4), "f32"
(exact, 4x slower PE).
"""

import numpy as np

import concourse.bass as bass
import concourse.mybir as mybir
import concourse.tile as tile
from concourse import bacc
from concourse.bass_utils import run_bass_kernel_spmd
from concourse.vector_clock import ScopedClock

F32 = mybir.dt.float32

NB, BIN, BOUT = 8, 512, 512
BATCH = 16384
NCORES = 8
P = 128
KT = BIN // P  # 4 k-tiles per block

SCHEME = "f16"  # "f16" | "f16_f32out" | "f32r" | "f32"

_patched = False


def _patch_tile_drain():
    """Walrus in this container accepts only one sync-wait per InstDrain;
    split the tile-exit drain's waits across one drain instruction each."""
    global _patched
    if _patched:
        return
    _patched = True

    def _drain_and_barrier(self, tick_clock, wait_clock):
        nc = self.nc
        drain_inst = nc.sync.drain()
        wait_clock.add_sem_waits(
            drain_inst.ins, ScopedClock({None: tick_clock.global_clock})
        )
        si = drain_inst.ins.sync_info
        if si is not None and len(si.on_wait) > 1:
            waits = list(si.on_wait)
            updates = list(si.on_update)
            drain_inst.ins.sync_info = mybir.SyncInfo(
                on_wait=[waits[0]], on_update=updates
            )
            for w in waits[1:]:
                extra = nc.sync.drain()
                extra.ins.sync_info = mybir.SyncInfo(on_wait=[w], on_update=[])
        nc.all_engine_barrier()
        popped = nc._tile_sem_poison_stack.pop()
        assert popped is self._sem_poison
        nc.clear_and_free_semaphores(list(self.sems.allocated().values()))
        nc.all_engine_barrier()

    tile.TileContext._drain_and_barrier = _drain_and_barrier


_nc_cache = {}


def _scheme_dtypes(scheme):
    if scheme in ("f16", "f16_f32out"):
        in_dt = mybir.dt.float16
        out_dt = mybir.dt.float16 if scheme == "f16" else F32
        np_in = np.float16
    elif scheme == "f32r":
        in_dt, out_dt, np_in = mybir.dt.float32r, F32, np.float32
    elif scheme == "f32":
        in_dt, out_dt, np_in = F32, F32, np.float32
    else:
        raise ValueError(scheme)
    return in_dt, out_dt, np_in


def _groups(group):
    """Batch-row group sizes: geometric ramp at the start (matmuls start on
    the first small tile and never starve while DMA builds runway) and small
    final groups (tail waits on a small final output DMA)."""
    head = [256, 512, 1024, 1536]
    tail = [1536, 1024, 128, 128]
    body = BATCH - sum(head) - sum(tail)
    sizes = head + [group] * (body // group)
    rem = body % group
    if rem:
        sizes.append(rem)
    sizes += tail
    assert sum(sizes) == BATCH, sizes
    return sizes


def _build(scheme=SCHEME, group=2048):
    key = (scheme, group)
    if key in _nc_cache:
        return _nc_cache[key]
    _patch_tile_drain()
    in_dt, out_dt, _ = _scheme_dtypes(scheme)

    nc = bacc.Bacc(None, target_bir_lowering=False)
    xT = nc.dram_tensor("xT", [BIN, BATCH], in_dt, kind="ExternalInput")
    wT = nc.dram_tensor("wT", [BIN, BOUT], in_dt, kind="ExternalInput")
    bias = nc.dram_tensor("bias", [P, BOUT], F32, kind="ExternalInput")
    y = nc.dram_tensor("y", [BATCH, BOUT], out_dt, kind="ExternalOutput")

    with tile.TileContext(nc) as tc:
        with (
            tc.tile_pool(name="const", bufs=1) as const,
            tc.tile_pool(name="xp", bufs=4) as xp,
            tc.tile_pool(name="yp", bufs=4) as yp,
            tc.tile_pool(name="ps", bufs=8, space="PSUM") as psp,
        ):
            # PE warmup: dummy fp32 matmuls with no DMA dependency keep the
            # PE busy during the DMA fill so the HAM clock-gate reaches
            # 2.4 GHz before the first real matmul (saves ~5us of cold MMs).
            scratch = const.tile([P, 640], F32)
            nc.gpsimd.memset(scratch[:], 0.0)
            warm_ps = psp.tile([P, BOUT], F32, tag="ps")
            for _ in range(4):
                nc.tensor.matmul(
                    warm_ps[:], scratch[:, :P], scratch[:, P:], start=True, stop=True
                )

            # W on the scalar HWDGE queue, x on sync: the two transfers run
            # on separate rings and overlap.
            wt = const.tile([P, KT, BOUT], in_dt)
            nc.scalar.dma_start(wt[:], wT.rearrange("(t p) o -> p t o", p=P))
            bt = const.tile([P, BOUT], F32)

            row = 0
            for mg, gsz in enumerate(_groups(group)):
                nsub = gsz // P
                xt = xp.tile([P, KT, gsz], in_dt, tag="xt")
                nc.sync.dma_start(
                    xt[:],
                    xT[:, row : row + gsz].rearrange("(t p) b -> p t b", p=P),
                )
                if mg == 0:
                    # bias is first needed by the first DVE add; load it
                    # after the first x tile so matmuls start sooner
                    nc.scalar.dma_start(bt[:], bias[:])
                yt = yp.tile([P, nsub, BOUT], out_dt, tag="yt")
                for ms in range(nsub):
                    ps = psp.tile([P, BOUT], F32, tag="ps")
                    for k in range(KT):
                        nc.tensor.matmul(
                            ps[:],
                            xt[:, k, ms * P : (ms + 1) * P],
                            wt[:, k, :],
                            start=(k == 0),
                            stop=(k == KT - 1),
                        )
                    nc.vector.tensor_add(out=yt[:, ms, :], in0=ps[:], in1=bt[:])
                nc.sync.dma_start(
                    y[row : row + gsz, :].rearrange("(s p) o -> p s o", p=P),
                    yt[:],
                )
                row += gsz
    nc.compile()
    _nc_cache[key] = nc
    return nc


LAST_RESULT = None


def kernel(x, W, b, trace=False, scheme=SCHEME, group=2048, trace_kwargs=None):
    global LAST_RESULT
    x = np.asarray(x, dtype=np.float32)
    W = np.asarray(W, dtype=np.float32)
    b = np.asarray(b, dtype=np.float32)

    _, _, np_in = _scheme_dtypes(scheme)
    nc = _build(scheme, group)
    in_maps = []
    for g in range(NCORES):
        xT_g = np.ascontiguousarray(x[:, g * BIN : (g + 1) * BIN].T.astype(np_in))
        wT_g = np.ascontiguousarray(W[g].T.astype(np_in))
        bias_g = np.ascontiguousarray(np.broadcast_to(b[g][None, :], (P, BOUT)))
        in_maps.append({"xT": xT_g, "wT": wT_g, "bias": bias_g})

    kwargs = dict(trace_kwargs or {})
    res = run_bass_kernel_spmd(nc, in_maps, list(range(NCORES)), trace=trace, **kwargs)
    LAST_RESULT = res

    out = np.empty((BATCH, NB * BOUT), dtype=np.float32)
    for g in range(NCORES):
        out[:, g * BOUT : (g + 1) * BOUT] = res.results[g]["y"].astype(np.float32)
    return out
